# revision 10
# baseline (speedup 1.0000x reference)
"""Trainium2 Bass kernel for nn_Explainer segment_reduce (cdist + bidirectional
segment min/mean) on 8 NeuronCores.

Math (reference):
    ef_n = (h[ne0] + h[ne1])/2, ef_l = (h[le0] + h[le1])/2
    M = -cdist(ef_n, ef_l)                      # [En, El]
    out_n = seg_mean_rows(seg_max_cols(M))      # [Gn, Gl]
    out_l = seg_mean_cols(seg_max_rows(M))      # [Gn, Gl]
    out = (out_n + out_l)/2

All maxes of M = -d become mins of d^2; sqrt/negate/mean run on host.

Device computes, per core (node segments LPT-packed onto cores):
    psum[m, c] = un_hat[m] . u_l[c] + bl2[c]        (PE)
      where un_hat = -0.5*(h[ne0]+h[ne1]) per row-slot (fp8 DoubleRow K=256)
      and bl2[c] = 0.25*|u_l[c]|^2 via K=1 ones x bl2 quadrant matmuls
    strip = psum + an2[m]  (= d^2, fp16)            (ACT, bias=0.25*|u_n|^2)
    row side: per label-segment min via width-class TT-min trees  (DVE)
    col side: running per-lane min across row tiles               (DVE)

Rows live in per-segment lane bands: segment s -> lanes [B_s, B_s+L_s),
row-tiles t in [0, nrt). Dummy slots duplicate the segment's first row, so
they are harmless for the col-side min and ignored by the host on the row
side. Label columns are padded per segment to a multiple of 32 with
duplicated edges (dups can't change a min; weight-masked out of means),
segments sorted by width so each width class gets a uniform strided tree.
Host: band-collapse col mins, sqrt, masked means, assemble [64, 64].
"""
import numpy as np
import ml_dtypes

import concourse.bacc as bacc
import concourse.tile as tile
import concourse.mybir as mybir
from concourse.alu_op_type import AluOpType
from concourse.bass_utils import run_bass_kernel_spmd

P = 128
N_CORES = 8
GN = GL = 64
D = 256
F16 = mybir.dt.float16
F32 = mybir.dt.float32
F8 = mybir.dt.float8e4
NP_F8 = ml_dtypes.float8_e4m3

USE_FP8 = False

_prog_cache = {}


def _build(nrt: int, cols_pad: int, classes: tuple, use_fp8: bool):
    ngrp = (cols_pad + 2047) // 2048

    nc = bacc.Bacc("TRN2", target_bir_lowering=False, debug=False,
                   num_devices=N_CORES)
    wdt = F8 if use_fp8 else F16
    ulT_in = nc.dram_tensor("ulT", [P, 2 * cols_pad], wdt, kind="ExternalInput")
    unT_in = nc.dram_tensor("unT", [P, nrt * 2 * P], wdt, kind="ExternalInput")
    bl2_in = nc.dram_tensor("bl2c", [4, cols_pad], F16, kind="ExternalInput")
    an2_in = nc.dram_tensor("an2q", [P, nrt], F32, kind="ExternalInput")
    rowout = nc.dram_tensor("rowout", [P, nrt * GL], F16, kind="ExternalOutput")
    collout = nc.dram_tensor("collout", [P, cols_pad], F16, kind="ExternalOutput")

    with tile.TileContext(nc) as tc:
        with (
            tc.tile_pool(name="persist", bufs=1) as pp,
            tc.tile_pool(name="strip", bufs=3) as sp,
            tc.tile_pool(name="small", bufs=2) as smp,
        ):
            u_lT = pp.tile([P, 2 * cols_pad], wdt, tag="u_lT")
            u_nT = pp.tile([P, nrt * 2 * P], wdt, tag="u_nT")
            bl2rep = pp.tile([97, cols_pad], F16, tag="bl2rep")
            an2q = pp.tile([P, nrt], F32, tag="an2q")
            coll = pp.tile([P, cols_pad], F16, tag="coll")
            onesrep = pp.tile([97, P], F16, tag="onesrep")

            u_lT3 = u_lT[:].rearrange("p (k c) -> p k c", k=2)
            u_nT4 = u_nT[:].rearrange("p (t k m) -> p t k m", t=nrt, k=2)

            # HAM warm-up: a dependency-free dense MM burst on scratch SBUF
            # during the input DMA window. The PE clock-gate only releases
            # (1.2 -> 2.4 GHz) after ~3.4us of gap-free matmul activity; the
            # real stream has per-group stalls and never qualifies on its own.
            wsrc = pp.tile([P, 512], F16, tag="warm_src")
            nc.gpsimd.memset(wsrc[:], 1.0)
            with tc.tile_pool(name="warmps", bufs=1, space="PSUM") as wps:
                wpt = wps.tile([P, 512], F32, tag="warm")
                for _ in range(24):
                    nc.tensor.matmul(wpt[:], wsrc[:, :P], wsrc[:],
                                     start=True, stop=True)

            nc.gpsimd.memset(onesrep[:], 1.0)
            nc.sync.dma_start(an2q[:], an2_in[:])
            nc.sync.dma_start(u_nT[:], unT_in[:])
            for srow in range(4):
                nc.sync.dma_start(bl2rep[32 * srow:32 * srow + 1, :],
                                  bl2_in[srow:srow + 1, :])
            for g in range(ngrp):
                g0 = g * 2048
                gw = min(2048, cols_pad - g0)
                for k in range(2):
                    nc.sync.dma_start(
                        u_lT[:, k * cols_pad + g0:k * cols_pad + g0 + gw],
                        ulT_in[:, k * cols_pad + g0:k * cols_pad + g0 + gw])

            with tc.tile_pool(name="ps", bufs=2, space="PSUM") as ps:
                for t in range(nrt):
                    strip = sp.tile([P, cols_pad], F16, tag="strip")
                    for g in range(ngrp):
                        g0 = g * 2048
                        gw = min(2048, cols_pad - g0)
                        pt = ps.tile([P, 2048], F32, tag="dot")
                        ns = gw // 512
                        if use_fp8:
                            for s in range(ns):
                                osl = slice(s * 512, (s + 1) * 512)
                                csl = slice(g0 + s * 512, g0 + (s + 1) * 512)
                                nc.tensor.matmul(
                                    pt[:, osl], u_nT4[:, t, :, :],
                                    u_lT3[:, :, csl], start=True, stop=False,
                                    perf_mode=mybir.MatmulPerfMode.DoubleRow)
                        else:
                            for k in range(2):
                                for s in range(ns):
                                    osl = slice(s * 512, (s + 1) * 512)
                                    csl = slice(g0 + s * 512, g0 + (s + 1) * 512)
                                    nc.tensor.matmul(
                                        pt[:, osl], u_nT4[:, t, k, :],
                                        u_lT3[:, k, csl], start=(k == 0),
                                        stop=False)
                        for s in range(ns):
                            osl = slice(s * 512, (s + 1) * 512)
                            csl = slice(g0 + s * 512, g0 + (s + 1) * 512)
                            pb = 32 * s
                            nc.tensor.matmul(
                                pt[:, osl], onesrep[pb:pb + 1, :],
                                bl2rep[pb:pb + 1, csl], start=False, stop=True,
                                tile_position=(pb, 0))
                        nc.scalar.activation(
                            strip[:, g0:g0 + gw], pt[:, :gw],
                            mybir.ActivationFunctionType.Identity,
                            bias=an2q[:, t:t + 1], scale=1.0)
                        # col side: running per-lane min
                        gsl = slice(g0, g0 + gw)
                        if t == 0:
                            nc.vector.tensor_copy(coll[:, gsl], strip[:, gsl])
                        else:
                            nc.vector.tensor_tensor(coll[:, gsl], coll[:, gsl],
                                                    strip[:, gsl],
                                                    op=AluOpType.min)
                            if t == nrt - 1:
                                nc.sync.dma_start(collout[:, gsl], coll[:, gsl])
                    # row side: per-class strided TT-min trees down to w=32
                    tacc = smp.tile([P, GL * 32], F16, tag="tacc")
                    tacc3 = tacc[:].rearrange("p (s w) -> p s w", w=32)
                    off_in = 0
                    off_out = 0
                    for (j, n_c) in classes:
                        span = n_c * 32 * j
                        v = strip[:, off_in:off_in + span].rearrange(
                            "p (s j w) -> p s j w", j=j, w=32)
                        tv = tacc3[:, off_out:off_out + n_c, :]
                        if j == 1:
                            nc.vector.tensor_copy(tv, v[:, :, 0, :])
                        else:
                            nc.vector.tensor_tensor(tv, v[:, :, 0, :],
                                                    v[:, :, 1, :],
                                                    op=AluOpType.min)
                            for jj in range(2, j):
                                nc.vector.tensor_tensor(tv, tv, v[:, :, jj, :],
                                                        op=AluOpType.min)
                        off_in += span
                        off_out += n_c
                    # final 32 -> 1 pairwise tree
                    cur = tacc
                    w = 32
                    while w > 1:
                        w //= 2
                        nxt = smp.tile([P, GL * w], F16, tag=f"red{w}")
                        v = cur[:].rearrange("p (s j w) -> p s j w", j=2, w=w)
                        nc.vector.tensor_tensor(
                            nxt[:].rearrange("p (s w) -> p s w", w=w),
                            v[:, :, 0, :], v[:, :, 1, :], op=AluOpType.min)
                        cur = nxt
                    nc.sync.dma_start(rowout[:, t * GL:(t + 1) * GL], cur[:])

    nc.compile()
    return nc


def _get_program(nrt, cols_pad, classes, use_fp8):
    key = (nrt, cols_pad, classes, use_fp8)
    if key not in _prog_cache:
        _prog_cache[key] = _build(nrt, cols_pad, classes, use_fp8)
    return _prog_cache[key]


def _assign_cores(cn, nrt):
    """LPT-pack segments onto cores by lane count; None if any core >128."""
    L = np.where(cn > 0, -(-cn // nrt), 0)
    order = np.argsort(-L, kind="stable")
    loads = [0] * N_CORES
    segs = [[] for _ in range(N_CORES)]
    for g in order:
        c = int(np.argmin(loads))
        loads[c] += int(L[g])
        segs[c].append(int(g))
    if max(loads) > P:
        return None
    return segs


def kernel(h, node_edge, node_batch, label_edge, label_batch):
    hf = np.asarray(h).astype(np.float32)
    ne = np.asarray(node_edge).astype(np.int64)
    nb = np.asarray(node_batch).astype(np.int64)
    le = np.asarray(label_edge).astype(np.int64)
    lb = np.asarray(label_batch).astype(np.int64)

    cn = np.bincount(nb, minlength=GN).astype(np.int64)
    cl = np.bincount(lb, minlength=GL).astype(np.int64)
    nb_off = np.concatenate([[0], np.cumsum(cn)])
    lb_off = np.concatenate([[0], np.cumsum(cl)])

    # ---- label columns: per-segment width = ceil(cl/32)*32, sorted by width
    w_seg = np.where(cl > 0, ((cl + 31) // 32) * 32, 32).astype(np.int64)
    lorder = np.argsort(w_seg, kind="stable")
    w_sorted = w_seg[lorder]
    offs = np.concatenate([[0], np.cumsum(w_sorted)])
    cols = int(offs[-1])
    cols_pad = -(-cols // 512) * 512
    classes = []
    for w in sorted(set(w_sorted.tolist())):
        classes.append((int(w) // 32, int((w_sorted == w).sum())))
    classes = tuple(classes)

    col_edge = np.zeros(cols_pad, np.int64)
    col_w = np.zeros(cols_pad, np.float64)
    for i in range(GL):
        g = int(lorder[i])
        w = int(w_sorted[i])
        c = int(cl[g])
        k = np.arange(w)
        if c > 0:
            col_edge[offs[i]:offs[i] + w] = lb_off[g] + (k % c)
            col_w[offs[i]:offs[i] + w] = (k < c).astype(np.float64)

    u_l = hf[le[0][col_edge]] + hf[le[1][col_edge]]          # [cols_pad, 256]
    bl2 = 0.25 * (u_l * u_l).sum(axis=1)                     # f32
    np_wdt = NP_F8 if USE_FP8 else np.float16
    ulT = np.ascontiguousarray(
        u_l.reshape(cols_pad, 2, P).transpose(2, 1, 0)
        .reshape(P, 2 * cols_pad)).astype(np_wdt)
    bl2c = np.ascontiguousarray(
        np.broadcast_to(bl2.astype(np.float16)[None, :], (4, cols_pad)))

    # ---- node rows: LPT pack segments onto cores, lane bands per segment
    u_n = hf[ne[0]] + hf[ne[1]]                              # [8192, 256]
    an2 = 0.25 * (u_n * u_n).sum(axis=1)                     # f32

    nrt = 8
    while True:
        segs_per_core = _assign_cores(cn, nrt)
        if segs_per_core is not None:
            break
        nrt += 1

    un_hat = (-0.5 * u_n).astype(np_wdt)                     # [8192, 256]

    in_maps = []
    core_slots = []
    for c in range(N_CORES):
        rows_glob = np.zeros((nrt, P), np.int64)
        slot_info = []
        B = 0
        for g in segs_per_core[c]:
            n_g = int(cn[g])
            if n_g == 0:
                slot_info.append((g, B, 0))
                continue
            Lg = -(-n_g // nrt)
            lanes_band = np.arange(Lg * nrt)
            lane = B + lanes_band // nrt
            ts = lanes_band % nrt
            # real rows j -> slot (lane[j], ts[j]); dummies duplicate row 0
            src = np.where(lanes_band < n_g, nb_off[g] + lanes_band, nb_off[g])
            rows_glob[ts, lane] = src
            slot_info.append((g, B, Lg))
            B += Lg
        unq = un_hat[rows_glob.reshape(-1)].reshape(nrt, P, 2, P)
        unT = np.ascontiguousarray(unq.transpose(3, 0, 2, 1)
                                   .reshape(P, nrt * 2 * P))
        an2q = np.ascontiguousarray(an2[rows_glob].T.astype(np.float32))
        in_maps.append({
            "ulT": ulT,
            "unT": unT,
            "bl2c": bl2c,
            "an2q": an2q,
        })
        core_slots.append(slot_info)

    nc = _get_program(nrt, cols_pad, classes, USE_FP8)
    res = run_bass_kernel_spmd(nc, in_maps, core_ids=list(range(N_CORES)))

    cl_sorted = cl[lorder]
    out_n = np.zeros((GN, GL), np.float64)
    out_l = np.zeros((GN, GL), np.float64)
    for c in range(N_CORES):
        r = res.results[c]
        rowe = r["rowout"].astype(np.float64).reshape(P, nrt, GL)
        colle = r["collout"].astype(np.float64)                 # [128, cols_pad]
        for (g, B, Lg) in core_slots[c]:
            n_g = int(cn[g])
            if n_g == 0:
                continue
            j = np.arange(n_g)
            lanes = B + j // nrt
            ts = j % nrt
            ev = rowe[lanes, ts, :]                             # [n_g, GL] sorted-label order
            dmin = np.sqrt(np.maximum(ev, 0.0))
            row_mean = np.zeros(GL)
            row_mean[lorder] = -dmin.mean(axis=0)
            row_mean[cl == 0] = 0.0
            out_n[g] = row_mean

            ecol = colle[B:B + Lg, :].min(axis=0)               # [cols_pad]
            dcol = np.sqrt(np.maximum(ecol, 0.0)) * col_w
            sums = np.add.reduceat(dcol, offs[:-1])
            col_mean = np.zeros(GL)
            col_mean[lorder] = -(sums / np.maximum(cl_sorted, 1))
            col_mean[cl == 0] = 0.0
            out_l[g] = col_mean

    return ((out_n + out_l) * 0.5).astype(np.float32)


# revision 11
# speedup vs baseline: 1.0488x; 1.0488x over previous
"""Trainium2 Bass kernel for nn_Explainer segment_reduce (cdist + bidirectional
segment min/mean) on 8 NeuronCores.

Math (reference):
    ef_n = (h[ne0] + h[ne1])/2, ef_l = (h[le0] + h[le1])/2
    M = -cdist(ef_n, ef_l)                      # [En, El]
    out_n = seg_mean_rows(seg_max_cols(M))      # [Gn, Gl]
    out_l = seg_mean_cols(seg_max_rows(M))      # [Gn, Gl]
    out = (out_n + out_l)/2

All maxes of M = -d become mins of d^2; sqrt/negate/mean run on host.

Device computes, per core (node segments LPT-packed onto cores):
    psum[m, c] = un_hat[m] . u_l[c] + bl2[c]        (PE)
      where un_hat = -0.5*(h[ne0]+h[ne1]) per row-slot (fp8 DoubleRow K=256)
      and bl2[c] = 0.25*|u_l[c]|^2 via K=1 ones x bl2 quadrant matmuls
    strip = psum + an2[m]  (= d^2, fp16)            (ACT, bias=0.25*|u_n|^2)
    row side: per label-segment min via width-class TT-min trees  (DVE)
    col side: running per-lane min across row tiles               (DVE)

Rows live in per-segment lane bands: segment s -> lanes [B_s, B_s+L_s),
row-tiles t in [0, nrt). Dummy slots duplicate the segment's first row, so
they are harmless for the col-side min and ignored by the host on the row
side. Label columns are padded per segment to a multiple of 32 with
duplicated edges (dups can't change a min; weight-masked out of means),
segments sorted by width so each width class gets a uniform strided tree.
Host: band-collapse col mins, sqrt, masked means, assemble [64, 64].
"""
import numpy as np
import ml_dtypes

import concourse.bacc as bacc
import concourse.tile as tile
import concourse.mybir as mybir
from concourse.alu_op_type import AluOpType
from concourse.bass_utils import run_bass_kernel_spmd

P = 128
N_CORES = 8
GN = GL = 64
D = 256
F16 = mybir.dt.float16
F32 = mybir.dt.float32
F8 = mybir.dt.float8e4
NP_F8 = ml_dtypes.float8_e4m3

USE_FP8 = True

_prog_cache = {}


def _build(nrt: int, cols_pad: int, classes: tuple, use_fp8: bool):
    ngrp = (cols_pad + 2047) // 2048

    nc = bacc.Bacc("TRN2", target_bir_lowering=False, debug=False,
                   num_devices=N_CORES)
    wdt = F8 if use_fp8 else F16
    ulT_in = nc.dram_tensor("ulT", [P, 2 * cols_pad], wdt, kind="ExternalInput")
    unT_in = nc.dram_tensor("unT", [P, nrt * 2 * P], wdt, kind="ExternalInput")
    bl2_in = nc.dram_tensor("bl2c", [4, cols_pad], F16, kind="ExternalInput")
    an2_in = nc.dram_tensor("an2q", [P, nrt], F32, kind="ExternalInput")
    rowout = nc.dram_tensor("rowout", [P, nrt * GL], F16, kind="ExternalOutput")
    collout = nc.dram_tensor("collout", [P, cols_pad], F16, kind="ExternalOutput")

    with tile.TileContext(nc) as tc:
        with (
            tc.tile_pool(name="persist", bufs=1) as pp,
            tc.tile_pool(name="strip", bufs=3) as sp,
            tc.tile_pool(name="small", bufs=2) as smp,
        ):
            u_lT = pp.tile([P, 2 * cols_pad], wdt, tag="u_lT")
            u_nT = pp.tile([P, nrt * 2 * P], wdt, tag="u_nT")
            bl2rep = pp.tile([97, cols_pad], F16, tag="bl2rep")
            an2q = pp.tile([P, nrt], F32, tag="an2q")
            coll = pp.tile([P, cols_pad], F16, tag="coll")
            onesrep = pp.tile([97, P], F16, tag="onesrep")

            u_lT3 = u_lT[:].rearrange("p (k c) -> p k c", k=2)
            u_nT4 = u_nT[:].rearrange("p (t k m) -> p t k m", t=nrt, k=2)

            # HAM warm-up: a dependency-free dense MM burst on scratch SBUF
            # during the input DMA window. The PE clock-gate only releases
            # (1.2 -> 2.4 GHz) after ~3.4us of gap-free matmul activity; the
            # real stream has per-group stalls and never qualifies on its own.
            wsrc = pp.tile([P, 512], F16, tag="warm_src")
            nc.gpsimd.memset(wsrc[:], 1.0)
            with tc.tile_pool(name="warmps", bufs=1, space="PSUM") as wps:
                wpt = wps.tile([P, 512], F32, tag="warm")
                for _ in range(24):
                    nc.tensor.matmul(wpt[:], wsrc[:, :P], wsrc[:],
                                     start=True, stop=True)

            nc.gpsimd.memset(onesrep[:], 1.0)
            nc.sync.dma_start(an2q[:], an2_in[:])
            nc.sync.dma_start(u_nT[:], unT_in[:])
            for srow in range(4):
                nc.sync.dma_start(bl2rep[32 * srow:32 * srow + 1, :],
                                  bl2_in[srow:srow + 1, :])
            for g in range(ngrp):
                g0 = g * 2048
                gw = min(2048, cols_pad - g0)
                for k in range(2):
                    nc.sync.dma_start(
                        u_lT[:, k * cols_pad + g0:k * cols_pad + g0 + gw],
                        ulT_in[:, k * cols_pad + g0:k * cols_pad + g0 + gw])

            with tc.tile_pool(name="ps", bufs=2, space="PSUM") as ps:
                for t in range(nrt):
                    strip = sp.tile([P, cols_pad], F16, tag="strip")
                    for g in range(ngrp):
                        g0 = g * 2048
                        gw = min(2048, cols_pad - g0)
                        pt = ps.tile([P, 2048], F32, tag="dot")
                        ns = gw // 512
                        if use_fp8:
                            for s in range(ns):
                                osl = slice(s * 512, (s + 1) * 512)
                                csl = slice(g0 + s * 512, g0 + (s + 1) * 512)
                                nc.tensor.matmul(
                                    pt[:, osl], u_nT4[:, t, :, :],
                                    u_lT3[:, :, csl], start=True, stop=False,
                                    perf_mode=mybir.MatmulPerfMode.DoubleRow)
                        else:
                            for k in range(2):
                                for s in range(ns):
                                    osl = slice(s * 512, (s + 1) * 512)
                                    csl = slice(g0 + s * 512, g0 + (s + 1) * 512)
                                    nc.tensor.matmul(
                                        pt[:, osl], u_nT4[:, t, k, :],
                                        u_lT3[:, k, csl], start=(k == 0),
                                        stop=False)
                        for s in range(ns):
                            osl = slice(s * 512, (s + 1) * 512)
                            csl = slice(g0 + s * 512, g0 + (s + 1) * 512)
                            pb = 32 * s
                            nc.tensor.matmul(
                                pt[:, osl], onesrep[pb:pb + 1, :],
                                bl2rep[pb:pb + 1, csl], start=False, stop=True,
                                tile_position=(pb, 0))
                        nc.scalar.activation(
                            strip[:, g0:g0 + gw], pt[:, :gw],
                            mybir.ActivationFunctionType.Identity,
                            bias=an2q[:, t:t + 1], scale=1.0)
                        # col side: running per-lane min
                        gsl = slice(g0, g0 + gw)
                        if t == 0:
                            nc.vector.tensor_copy(coll[:, gsl], strip[:, gsl])
                        else:
                            nc.vector.tensor_tensor(coll[:, gsl], coll[:, gsl],
                                                    strip[:, gsl],
                                                    op=AluOpType.min)
                            if t == nrt - 1:
                                nc.sync.dma_start(collout[:, gsl], coll[:, gsl])
                    # row side: per-class strided TT-min trees down to w=32
                    tacc = smp.tile([P, GL * 32], F16, tag="tacc")
                    tacc3 = tacc[:].rearrange("p (s w) -> p s w", w=32)
                    off_in = 0
                    off_out = 0
                    for (j, n_c) in classes:
                        span = n_c * 32 * j
                        v = strip[:, off_in:off_in + span].rearrange(
                            "p (s j w) -> p s j w", j=j, w=32)
                        tv = tacc3[:, off_out:off_out + n_c, :]
                        if j == 1:
                            nc.vector.tensor_copy(tv, v[:, :, 0, :])
                        else:
                            nc.vector.tensor_tensor(tv, v[:, :, 0, :],
                                                    v[:, :, 1, :],
                                                    op=AluOpType.min)
                            for jj in range(2, j):
                                nc.vector.tensor_tensor(tv, tv, v[:, :, jj, :],
                                                        op=AluOpType.min)
                        off_in += span
                        off_out += n_c
                    # final 32 -> 1 pairwise tree
                    cur = tacc
                    w = 32
                    while w > 1:
                        w //= 2
                        nxt = smp.tile([P, GL * w], F16, tag=f"red{w}")
                        v = cur[:].rearrange("p (s j w) -> p s j w", j=2, w=w)
                        nc.vector.tensor_tensor(
                            nxt[:].rearrange("p (s w) -> p s w", w=w),
                            v[:, :, 0, :], v[:, :, 1, :], op=AluOpType.min)
                        cur = nxt
                    nc.sync.dma_start(rowout[:, t * GL:(t + 1) * GL], cur[:])

    nc.compile()
    return nc


def _get_program(nrt, cols_pad, classes, use_fp8):
    key = (nrt, cols_pad, classes, use_fp8)
    if key not in _prog_cache:
        _prog_cache[key] = _build(nrt, cols_pad, classes, use_fp8)
    return _prog_cache[key]


def _assign_cores(cn, nrt):
    """LPT-pack segments onto cores by lane count; None if any core >128."""
    L = np.where(cn > 0, -(-cn // nrt), 0)
    order = np.argsort(-L, kind="stable")
    loads = [0] * N_CORES
    segs = [[] for _ in range(N_CORES)]
    for g in order:
        c = int(np.argmin(loads))
        loads[c] += int(L[g])
        segs[c].append(int(g))
    if max(loads) > P:
        return None
    return segs


def kernel(h, node_edge, node_batch, label_edge, label_batch):
    hf = np.asarray(h).astype(np.float32)
    ne = np.asarray(node_edge).astype(np.int64)
    nb = np.asarray(node_batch).astype(np.int64)
    le = np.asarray(label_edge).astype(np.int64)
    lb = np.asarray(label_batch).astype(np.int64)

    cn = np.bincount(nb, minlength=GN).astype(np.int64)
    cl = np.bincount(lb, minlength=GL).astype(np.int64)
    nb_off = np.concatenate([[0], np.cumsum(cn)])
    lb_off = np.concatenate([[0], np.cumsum(cl)])

    # ---- label columns: per-segment width = ceil(cl/32)*32, sorted by width
    w_seg = np.where(cl > 0, ((cl + 31) // 32) * 32, 32).astype(np.int64)
    lorder = np.argsort(w_seg, kind="stable")
    w_sorted = w_seg[lorder]
    offs = np.concatenate([[0], np.cumsum(w_sorted)])
    cols = int(offs[-1])
    cols_pad = -(-cols // 512) * 512
    classes = []
    for w in sorted(set(w_sorted.tolist())):
        classes.append((int(w) // 32, int((w_sorted == w).sum())))
    classes = tuple(classes)

    col_edge = np.zeros(cols_pad, np.int64)
    col_w = np.zeros(cols_pad, np.float64)
    for i in range(GL):
        g = int(lorder[i])
        w = int(w_sorted[i])
        c = int(cl[g])
        k = np.arange(w)
        if c > 0:
            col_edge[offs[i]:offs[i] + w] = lb_off[g] + (k % c)
            col_w[offs[i]:offs[i] + w] = (k < c).astype(np.float64)

    u_l = hf[le[0][col_edge]] + hf[le[1][col_edge]]          # [cols_pad, 256]
    bl2 = 0.25 * (u_l * u_l).sum(axis=1)                     # f32
    np_wdt = NP_F8 if USE_FP8 else np.float16
    ulT = np.ascontiguousarray(
        u_l.reshape(cols_pad, 2, P).transpose(2, 1, 0)
        .reshape(P, 2 * cols_pad)).astype(np_wdt)
    bl2c = np.ascontiguousarray(
        np.broadcast_to(bl2.astype(np.float16)[None, :], (4, cols_pad)))

    # ---- node rows: LPT pack segments onto cores, lane bands per segment
    u_n = hf[ne[0]] + hf[ne[1]]                              # [8192, 256]
    an2 = 0.25 * (u_n * u_n).sum(axis=1)                     # f32

    nrt = 8
    while True:
        segs_per_core = _assign_cores(cn, nrt)
        if segs_per_core is not None:
            break
        nrt += 1

    un_hat = (-0.5 * u_n).astype(np_wdt)                     # [8192, 256]

    in_maps = []
    core_slots = []
    for c in range(N_CORES):
        rows_glob = np.zeros((nrt, P), np.int64)
        slot_info = []
        B = 0
        for g in segs_per_core[c]:
            n_g = int(cn[g])
            if n_g == 0:
                slot_info.append((g, B, 0))
                continue
            Lg = -(-n_g // nrt)
            lanes_band = np.arange(Lg * nrt)
            lane = B + lanes_band // nrt
            ts = lanes_band % nrt
            # real rows j -> slot (lane[j], ts[j]); dummies duplicate row 0
            src = np.where(lanes_band < n_g, nb_off[g] + lanes_band, nb_off[g])
            rows_glob[ts, lane] = src
            slot_info.append((g, B, Lg))
            B += Lg
        unq = un_hat[rows_glob.reshape(-1)].reshape(nrt, P, 2, P)
        unT = np.ascontiguousarray(unq.transpose(3, 0, 2, 1)
                                   .reshape(P, nrt * 2 * P))
        an2q = np.ascontiguousarray(an2[rows_glob].T.astype(np.float32))
        in_maps.append({
            "ulT": ulT,
            "unT": unT,
            "bl2c": bl2c,
            "an2q": an2q,
        })
        core_slots.append(slot_info)

    nc = _get_program(nrt, cols_pad, classes, USE_FP8)
    res = run_bass_kernel_spmd(nc, in_maps, core_ids=list(range(N_CORES)))

    cl_sorted = cl[lorder]
    out_n = np.zeros((GN, GL), np.float64)
    out_l = np.zeros((GN, GL), np.float64)
    for c in range(N_CORES):
        r = res.results[c]
        rowe = r["rowout"].astype(np.float64).reshape(P, nrt, GL)
        colle = r["collout"].astype(np.float64)                 # [128, cols_pad]
        for (g, B, Lg) in core_slots[c]:
            n_g = int(cn[g])
            if n_g == 0:
                continue
            j = np.arange(n_g)
            lanes = B + j // nrt
            ts = j % nrt
            ev = rowe[lanes, ts, :]                             # [n_g, GL] sorted-label order
            dmin = np.sqrt(np.maximum(ev, 0.0))
            row_mean = np.zeros(GL)
            row_mean[lorder] = -dmin.mean(axis=0)
            row_mean[cl == 0] = 0.0
            out_n[g] = row_mean

            ecol = colle[B:B + Lg, :].min(axis=0)               # [cols_pad]
            dcol = np.sqrt(np.maximum(ecol, 0.0)) * col_w
            sums = np.add.reduceat(dcol, offs[:-1])
            col_mean = np.zeros(GL)
            col_mean[lorder] = -(sums / np.maximum(cl_sorted, 1))
            col_mean[cl == 0] = 0.0
            out_l[g] = col_mean

    return ((out_n + out_l) * 0.5).astype(np.float32)
